# revision 1
# baseline (speedup 1.0000x reference)
"""AttentionFuserV3 Trainium2 kernel: 8-core pure data parallel over batch.

Reference computation per batch item x_b [L=1024, D=512]:
  stage1: q = x W1^T; S = q x^T; A = softmax(S); mix = A x;
          h = tanh([mix, q] Wo1^T); h = h / max(||h||_2, eps)     (per row)
  stage2: c = [h, x]; q2 = c W2^T; S2 = q2 c^T; A2 = softmax(S2);
          mix2 = A2 c; o = [mix2, q2] Wo2^T; emb = mean_l(o)

Layout strategy ("T-space"): all big tensors are kept transposed in SBUF
(feature dim on partitions, sequence dim L on the free axis) so every
matmul contraction lands on the partition axis without on-device
transposes of the attention matrix.  Softmax runs without max-subtraction
(|scores| < ~70, exp stays in f32 range); the denominator is accumulated
with a ones-vector matmul and applied as a column broadcast produced by a
rank-1 matmul.  Only hidden_norm needs an on-device transpose back to
natural layout (PE transpose, 32 tiles/batch), spilled through DRAM.

Matmuls run in float32r (full PE speed at N=512, ~13-bit mantissa).
"""

import sys

sys.path.insert(0, "/opt/trn_rl_repo")

import numpy as np

N_GLOBAL, L, D = 32, 1024, 512
NCORES = 8
B = N_GLOBAL // NCORES          # 4 batch items per core
P = 128
LC = 512                        # l-chunk (matmul moving free dim)
NLC = L // LC                   # 2
DT = D // P                     # 4
LT = L // P                     # 8
D2T = 2 * D // P                # 8
C2T = 4 * D // P                # 16

_CACHE = {}


def _build_nc():
    import concourse.bass as bass  # noqa: F401
    import concourse.mybir as mybir
    import concourse.tile as tile
    from concourse import bacc

    f32 = mybir.dt.float32
    f32r = mybir.dt.float32r
    AF = mybir.ActivationFunctionType
    ALU = mybir.AluOpType

    nc = bacc.Bacc("TRN2", target_bir_lowering=False, debug=False,
                   num_devices=NCORES)

    x_ext = nc.declare_dram_parameter("x", [B, L, D], f32r, isOutput=False)
    xT_ext = nc.declare_dram_parameter("xT", [B, D, L], f32r, isOutput=False)
    w1t_ext = nc.declare_dram_parameter("w1t", [D, D], f32r, isOutput=False)
    wo1t_ext = nc.declare_dram_parameter("wo1t", [2 * D, D], f32r, isOutput=False)
    w2t_ext = nc.declare_dram_parameter("w2t", [2 * D, 2 * D], f32r, isOutput=False)
    wo2t_ext = nc.declare_dram_parameter("wo2t", [4 * D, D], f32r, isOutput=False)
    # Constants shipped from host: walrus's ISA check rejects memset/iota
    # writes into float32r tiles, but DMA from an f32r DRAM param is fine.
    id_ext = nc.declare_dram_parameter("ident", [P, P], f32r, isOutput=False)
    onc_ext = nc.declare_dram_parameter("onesc", [P, 1], f32r, isOutput=False)
    onr_ext = nc.declare_dram_parameter("onesr", [1, P], f32r, isOutput=False)
    out_ext = nc.declare_dram_parameter("out", [B, D], f32, isOutput=True)

    import time as _time
    _t0 = _time.time()
    with tile.TileContext(nc) as tc:
        with tc.tile_pool(name="wp", bufs=1) as wp, \
             tc.tile_pool(name="cp", bufs=1) as cp, \
             tc.tile_pool(name="xp", bufs=1) as xp, \
             tc.tile_pool(name="hp", bufs=1) as hp, \
             tc.tile_pool(name="tp", bufs=1) as tp, \
             tc.tile_pool(name="vp", bufs=2) as vp, \
             tc.tile_pool(name="ep", bufs=1) as ep, \
             tc.tile_pool(name="ps", bufs=8, space="PSUM") as pp:

            # ---- weights (resident) ----
            w1t_s = wp.tile([P, DT, D], f32r, tag="w1t")
            nc.sync.dma_start(out=w1t_s, in_=w1t_ext.rearrange("(k p) e -> p k e", p=P))
            wo1t_s = wp.tile([P, D2T, D], f32r, tag="wo1t")
            nc.sync.dma_start(out=wo1t_s, in_=wo1t_ext.rearrange("(k p) e -> p k e", p=P))
            w2t_s = wp.tile([P, D2T, 2 * D], f32r, tag="w2t")
            nc.sync.dma_start(out=w2t_s, in_=w2t_ext.rearrange("(k p) e -> p k e", p=P))
            wo2t_s = wp.tile([P, C2T, D], f32r, tag="wo2t")
            nc.sync.dma_start(out=wo2t_s, in_=wo2t_ext.rearrange("(k p) e -> p k e", p=P))

            # ---- constants (DMA'd from host; see note at declarations) ----
            ident_s = cp.tile([P, P], f32r, tag="ident")
            nc.sync.dma_start(out=ident_s, in_=id_ext[:, :])
            ones_s = cp.tile([P, 1], f32r, tag="ones")
            nc.sync.dma_start(out=ones_s, in_=onc_ext[:, :])
            onesr_s = cp.tile([1, P], f32r, tag="onesr")
            nc.sync.dma_start(out=onesr_s, in_=onr_ext[:, :])

            emb_s = ep.tile([P, B, DT], f32, tag="emb")

            def mm(out, lhsT, rhs, first, last):
                nc.tensor.matmul(out, lhsT, rhs, start=first, stop=last)

            def bcast_recip(denom_ps, clamp_eps=None):
                """[1,512] PSUM denominator -> [128,512] SBUF broadcast of its
                reciprocal (optionally sqrt+clamp first)."""
                rv = vp.tile([1, LC], f32r, tag="rv", bufs=1)
                with nc.allow_low_precision(reason="f32r rounding of softmax scale"):
                    if clamp_eps is not None:
                        nv = vp.tile([1, LC], f32, tag="nv", bufs=1)
                        nc.scalar.sqrt(nv, denom_ps[0:1, :])
                        nc.vector.tensor_scalar_max(nv, nv, clamp_eps)
                        nc.vector.reciprocal(rv, nv)
                    else:
                        nc.vector.reciprocal(rv, denom_ps[0:1, :])
                ps_b = pp.tile([P, LC], f32, tag="ps")
                mm(ps_b, onesr_s, rv[0:1, :], True, True)
                bc = vp.tile([P, LC], f32, tag="bc", bufs=1)
                nc.scalar.copy(bc, ps_b)
                return bc

            for b in range(B):
                xT_s = xp.tile([P, DT, L], f32r, tag="xT")
                nc.sync.dma_start(out=xT_s, in_=xT_ext[b].rearrange("(k p) l -> p k l", p=P))
                x_s = xp.tile([P, LT, D], f32r, tag="x")
                nc.sync.dma_start(out=x_s, in_=x_ext[b].rearrange("(k p) d -> p k d", p=P))
                hTn_s = hp.tile([P, DT, L], f32r, tag="hTn")
                red_s = vp.tile([P, DT, NLC], f32, tag="red")

                # ================= stage 1 =================
                for lc in range(NLC):
                    ls = slice(lc * LC, (lc + 1) * LC)

                    # ph1: qT[e,l] = W1T^T-contraction over d
                    qT_s = tp.tile([P, DT, LC], f32r, tag="qt")
                    for et in range(DT):
                        ps = pp.tile([P, LC], f32, tag="ps")
                        for dk in range(DT):
                            mm(ps, w1t_s[:, dk, et * P:(et + 1) * P],
                               xT_s[:, dk, ls], dk == 0, dk == DT - 1)
                        nc.scalar.copy(qT_s[:, et, :], ps)

                    # ph2: scoresT[m,l] -> exp -> denom
                    expT_s = tp.tile([P, LT, LC], f32r, tag="exp")
                    ps_d = pp.tile([P, LC], f32, tag="ps")
                    for mt in range(LT):
                        ps = pp.tile([P, LC], f32, tag="ps")
                        for ek in range(DT):
                            mm(ps, xT_s[:, ek, mt * P:(mt + 1) * P],
                               qT_s[:, ek, :], ek == 0, ek == DT - 1)
                        nc.scalar.activation(expT_s[:, mt, :], ps, AF.Exp)
                        mm(ps_d[0:1, :], ones_s, expT_s[:, mt, :],
                           mt == 0, mt == LT - 1)
                    bc1 = bcast_recip(ps_d)

                    # ph3: mixT'[d,l] = sum_m x[m,d] expT[m,l], then normalize
                    mixT_s = tp.tile([P, DT, LC], f32r, tag="mix")
                    ps_m = [pp.tile([P, LC], f32, tag="ps", name=f"psm_{b}_{lc}_{i}")
                            for i in range(DT)]
                    for mk in range(LT):
                        for dt in range(DT):
                            mm(ps_m[dt], x_s[:, mk, dt * P:(dt + 1) * P],
                               expT_s[:, mk, :], mk == 0, mk == LT - 1)
                    for dt in range(DT):
                        nc.vector.tensor_mul(mixT_s[:, dt, :], ps_m[dt], bc1)

                    # ph4: out1T[o,l] = Wo1T-contraction over c=[mix,q]; tanh
                    hT_s = tp.tile([P, DT, LC], f32, tag="ht")
                    for ot in range(DT):
                        ps = pp.tile([P, LC], f32, tag="ps")
                        for ck in range(D2T):
                            rhs = mixT_s[:, ck, :] if ck < DT else qT_s[:, ck - DT, :]
                            mm(ps, wo1t_s[:, ck, ot * P:(ot + 1) * P],
                               rhs, ck == 0, ck == D2T - 1)
                        nc.scalar.activation(hT_s[:, ot, :], ps, AF.Tanh)

                    # ph5: L2 norm over d (partition axis) via ones-matmul
                    hsq_s = tp.tile([P, DT, LC], f32r, tag="qt")
                    for dt in range(DT):
                        nc.scalar.activation(hsq_s[:, dt, :], hT_s[:, dt, :], AF.Square)
                    ps_n = pp.tile([P, LC], f32, tag="ps")
                    for dt in range(DT):
                        mm(ps_n[0:1, :], ones_s, hsq_s[:, dt, :], dt == 0, dt == DT - 1)
                    bc2 = bcast_recip(ps_n, clamp_eps=1e-12)
                    for dt in range(DT):
                        nc.vector.tensor_mul(hTn_s[:, dt, ls], hT_s[:, dt, :], bc2)

                # ph6: transpose hidden_norm back to natural layout.
                # Reuses the now-dead stage-1 qt/ht pool slots for hn.
                hn_a = tp.tile([P, LT // 2, D], f32r, tag="qt", name=f"hna_{b}")
                hn_b = tp.tile([P, LT // 2, D], f32r, tag="ht", name=f"hnb_{b}")

                def hn_nat(lt):
                    return hn_a[:, lt, :] if lt < LT // 2 else hn_b[:, lt - LT // 2, :]

                for lt in range(LT):
                    for dt in range(DT):
                        ps_t = pp.tile([P, P], f32r, tag="ps", name=f"pst_{b}_{lt}_{dt}")
                        nc.tensor.transpose(
                            ps_t, hTn_s[:, dt, lt * P:(lt + 1) * P], ident_s)
                        nc.scalar.copy(hn_nat(lt)[:, dt * P:(dt + 1) * P], ps_t)

                # ================= stage 2 =================
                for lc in range(NLC):
                    ls = slice(lc * LC, (lc + 1) * LC)

                    def c2T(k, fslice):
                        """combined2T[d2,·] k-tile: [hTn; xT]"""
                        return (hTn_s[:, k, fslice] if k < DT
                                else xT_s[:, k - DT, fslice])

                    # ph7: q2T[e2,l]
                    q2T_s = tp.tile([P, D2T, LC], f32r, tag="q2")
                    for et in range(D2T):
                        ps = pp.tile([P, LC], f32, tag="ps")
                        for dk in range(D2T):
                            mm(ps, w2t_s[:, dk, et * P:(et + 1) * P],
                               c2T(dk, ls), dk == 0, dk == D2T - 1)
                        nc.scalar.copy(q2T_s[:, et, :], ps)

                    # ph8: scores2T -> exp2 -> denom2
                    exp2T_s = tp.tile([P, LT, LC], f32r, tag="exp")
                    ps_d = pp.tile([P, LC], f32, tag="ps")
                    for mt in range(LT):
                        ps = pp.tile([P, LC], f32, tag="ps")
                        for ek in range(D2T):
                            mm(ps, c2T(ek, slice(mt * P, (mt + 1) * P)),
                               q2T_s[:, ek, :], ek == 0, ek == D2T - 1)
                        nc.scalar.activation(exp2T_s[:, mt, :], ps, AF.Exp)
                        mm(ps_d[0:1, :], ones_s, exp2T_s[:, mt, :],
                           mt == 0, mt == LT - 1)
                    bc3 = bcast_recip(ps_d)

                    # ph9: mix2T'[d2,l] = sum_m [hn, x][m,d2] exp2T[m,l]
                    mix2T_s = tp.tile([P, D2T, LC], f32r, tag="mix")
                    ps_m = [pp.tile([P, LC], f32, tag="ps", name=f"psm2_{b}_{lc}_{i}")
                            for i in range(D2T)]
                    for mk in range(LT):
                        for dt in range(D2T):
                            lhsT = (hn_nat(mk)[:, dt * P:(dt + 1) * P] if dt < DT
                                    else x_s[:, mk, (dt - DT) * P:(dt - DT + 1) * P])
                            mm(ps_m[dt], lhsT, exp2T_s[:, mk, :],
                               mk == 0, mk == LT - 1)
                    for dt in range(D2T):
                        nc.vector.tensor_mul(mix2T_s[:, dt, :], ps_m[dt], bc3)

                    # ph10: out2T[o,l]; mean over l (free axis)
                    for ot in range(DT):
                        ps = pp.tile([P, LC], f32, tag="ps")
                        for ck in range(C2T):
                            rhs = (mix2T_s[:, ck, :] if ck < D2T
                                   else q2T_s[:, ck - D2T, :])
                            mm(ps, wo2t_s[:, ck, ot * P:(ot + 1) * P],
                               rhs, ck == 0, ck == C2T - 1)
                        nc.vector.tensor_reduce(
                            red_s[:, ot, lc:lc + 1], ps,
                            axis=mybir.AxisListType.X, op=ALU.add)

                nc.vector.tensor_reduce(emb_s[:, b, :], red_s,
                                        axis=mybir.AxisListType.X, op=ALU.add)
                nc.scalar.mul(emb_s[:, b, :], emb_s[:, b, :], 1.0 / L)

            nc.sync.dma_start(out=out_ext.rearrange("b (t p) -> p b t", p=P),
                              in_=emb_s)

    _t1 = _time.time()
    nc.compile()
    print(f"[kernel] tile-trace+schedule {_t1 - _t0:.1f}s, "
          f"bacc compile {_time.time() - _t1:.1f}s", file=sys.stderr, flush=True)
    return nc


def get_nc():
    if "nc" not in _CACHE:
        _CACHE["nc"] = _build_nc()
    return _CACHE["nc"]


def make_in_maps(x, W1, Wo1, W2, Wo2):
    x = np.ascontiguousarray(np.asarray(x, dtype=np.float32))
    xT = np.ascontiguousarray(x.transpose(0, 2, 1))
    w1t = np.ascontiguousarray(np.asarray(W1, np.float32).T)
    wo1t = np.ascontiguousarray(np.asarray(Wo1, np.float32).T)
    w2t = np.ascontiguousarray(np.asarray(W2, np.float32).T)
    wo2t = np.ascontiguousarray(np.asarray(Wo2, np.float32).T)
    ident = np.eye(P, dtype=np.float32)
    onesc = np.ones((P, 1), dtype=np.float32)
    onesr = np.ones((1, P), dtype=np.float32)
    return [
        {"x": x[c * B:(c + 1) * B], "xT": xT[c * B:(c + 1) * B],
         "w1t": w1t, "wo1t": wo1t, "w2t": w2t, "wo2t": wo2t,
         "ident": ident, "onesc": onesc, "onesr": onesr}
        for c in range(NCORES)
    ]


def run(x, W1, Wo1, W2, Wo2, trace=False, **kw):
    from concourse.bass_utils import run_bass_kernel_spmd
    nc = get_nc()
    in_maps = make_in_maps(x, W1, Wo1, W2, Wo2)
    res = run_bass_kernel_spmd(nc, in_maps, core_ids=list(range(NCORES)),
                               trace=trace, **kw)
    out = np.concatenate([res.results[c]["out"] for c in range(NCORES)], axis=0)
    return out.reshape(N_GLOBAL, D, 1, 1), res


def kernel(**inputs):
    out, _ = run(inputs["x"], inputs["W1"], inputs["Wo1"],
                 inputs["W2"], inputs["Wo2"])
    return out



# revision 33
# speedup vs baseline: 1.1160x; 1.1160x over previous
"""AttentionFuserV3 Trainium2 kernel: 8-core pure data parallel over batch.

Reference computation per batch item x_b [L=1024, D=512]:
  stage1: q = x W1^T; S = q x^T; A = softmax(S); mix = A x;
          h = tanh([mix, q] Wo1^T); h = h / max(||h||_2, eps)     (per row)
  stage2: c = [h, x]; q2 = c W2^T; S2 = q2 c^T; A2 = softmax(S2);
          mix2 = A2 c; o = [mix2, q2] Wo2^T; emb = mean_l(o)

Pooling algebra: emb = mean_l(o) is linear, so the full [L,2D] mix2 and
[L,D] output projection are never materialized.  Instead
  emb = (1/L) [colsum(A2) c, colsum(q2)] Wo2^T
where colsum(A2)[m] = sum_l exp(S2[l,m]) / denom[l] is a cheap fused
multiply-reduce over the already-computed exp tiles.  This removes the
two largest matmul groups of stage 2 (mix2 and the output projection)
and the stage-1->stage-2 transpose of hidden_norm.

Layout strategy ("T-space"): all big tensors are kept transposed in SBUF
(feature dim on partitions, sequence dim L on the free axis) so every
matmul contraction lands on the partition axis without on-device
transposes of the attention matrix.  Softmax runs without max-subtraction
(|scores| < ~70, exp stays in f32 range); the denominator is accumulated
with a ones-vector matmul and applied as a column broadcast produced by a
rank-1 matmul.  Only hidden_norm needs an on-device transpose back to
natural layout (PE transpose, 32 tiles/batch), spilled through DRAM.

Matmuls run in float32r (full PE speed at N=512, ~13-bit mantissa).
"""

import sys

sys.path.insert(0, "/opt/trn_rl_repo")

import numpy as np

N_GLOBAL, L, D = 32, 1024, 512
NCORES = 8
B = N_GLOBAL // NCORES          # 4 batch items per core
P = 128
LC = 512                        # l-chunk (matmul moving free dim)
NLC = L // LC                   # 2
DT = D // P                     # 4
LT = L // P                     # 8
D2T = 2 * D // P                # 8
C2T = 4 * D // P                # 16

_CACHE = {}


def _build_nc():
    import concourse.bass as bass  # noqa: F401
    import concourse.mybir as mybir
    import concourse.tile as tile
    from concourse import bacc

    f32 = mybir.dt.float32
    f32r = mybir.dt.float32r
    AF = mybir.ActivationFunctionType
    ALU = mybir.AluOpType

    nc = bacc.Bacc("TRN2", target_bir_lowering=False, debug=False,
                   num_devices=NCORES)

    x_ext = nc.declare_dram_parameter("x", [B, L, D], f32r, isOutput=False)
    xT_ext = nc.declare_dram_parameter("xT", [B, D, L], f32r, isOutput=False)
    w1t_ext = nc.declare_dram_parameter("w1t", [D, D], f32r, isOutput=False)
    wo1t_ext = nc.declare_dram_parameter("wo1t", [2 * D, D], f32r, isOutput=False)
    w2t_ext = nc.declare_dram_parameter("w2t", [2 * D, 2 * D], f32r, isOutput=False)
    wo2t_ext = nc.declare_dram_parameter("wo2t", [4 * D, D], f32r, isOutput=False)
    # Constants shipped from host: walrus's ISA check rejects memset/iota
    # writes into float32r tiles, but DMA from an f32r DRAM param is fine.
    id_ext = nc.declare_dram_parameter("ident", [P, P], f32r, isOutput=False)
    onc_ext = nc.declare_dram_parameter("onesc", [P, 1], f32r, isOutput=False)
    onr_ext = nc.declare_dram_parameter("onesr", [1, P], f32r, isOutput=False)
    # DRAM bounce buffer for the pooled-mix row -> column transpose
    mscr_ext = nc.declare_dram_parameter("mscr", [B, 2 * D], f32, isOutput=True)
    out_ext = nc.declare_dram_parameter("out", [B, D], f32, isOutput=True)

    import time as _time
    _t0 = _time.time()
    with tile.TileContext(nc) as tc:
        with tc.tile_pool(name="wp", bufs=1) as wp, \
             tc.tile_pool(name="cp", bufs=1) as cp, \
             tc.tile_pool(name="xp", bufs=1) as xp, \
             tc.tile_pool(name="hp", bufs=1) as hp, \
             tc.tile_pool(name="tp", bufs=1) as tp, \
             tc.tile_pool(name="vp", bufs=2) as vp, \
             tc.tile_pool(name="ep", bufs=1) as ep, \
             tc.tile_pool(name="ps", bufs=8, space="PSUM") as pp:

            # ---- weights (resident) ----
            w1t_s = wp.tile([P, DT, D], f32r, tag="w1t")
            nc.sync.dma_start(out=w1t_s, in_=w1t_ext.rearrange("(k p) e -> p k e", p=P))
            wo1t_s = wp.tile([P, D2T, D], f32r, tag="wo1t")
            nc.sync.dma_start(out=wo1t_s, in_=wo1t_ext.rearrange("(k p) e -> p k e", p=P))
            w2t_s = wp.tile([P, D2T, 2 * D], f32r, tag="w2t")
            nc.sync.dma_start(out=w2t_s, in_=w2t_ext.rearrange("(k p) e -> p k e", p=P))
            wo2t_s = wp.tile([P, C2T, D], f32r, tag="wo2t")
            nc.sync.dma_start(out=wo2t_s, in_=wo2t_ext.rearrange("(k p) e -> p k e", p=P))

            # ---- constants (DMA'd from host; see note at declarations) ----
            ident_s = cp.tile([P, P], f32r, tag="ident")
            nc.sync.dma_start(out=ident_s, in_=id_ext[:, :])
            ones_s = cp.tile([P, 1], f32r, tag="ones")
            nc.sync.dma_start(out=ones_s, in_=onc_ext[:, :])
            onesr_s = cp.tile([1, P], f32r, tag="onesr")
            nc.sync.dma_start(out=onesr_s, in_=onr_ext[:, :])




            def mm(out, lhsT, rhs, first, last):
                nc.tensor.matmul(out, lhsT, rhs, start=first, stop=last)

            def bcast_recip(denom_ps, clamp_eps=None):
                """[1,512] PSUM denominator -> [128,512] SBUF broadcast of its
                reciprocal (optionally sqrt+clamp first)."""
                rv = vp.tile([1, LC], f32r, tag="rv", bufs=1)
                with nc.allow_low_precision(reason="f32r rounding of softmax scale"):
                    if clamp_eps is not None:
                        nv = vp.tile([1, LC], f32, tag="nv", bufs=1)
                        nc.scalar.sqrt(nv, denom_ps[0:1, :])
                        nc.vector.tensor_scalar_max(nv, nv, clamp_eps)
                        nc.vector.reciprocal(rv, nv)
                    else:
                        nc.vector.reciprocal(rv, denom_ps[0:1, :])
                ps_b = pp.tile([P, LC], f32, tag="ps")
                mm(ps_b, onesr_s, rv[0:1, :], True, True)
                bc = vp.tile([P, LC], f32, tag="bc", bufs=1)
                nc.scalar.copy(bc, ps_b)
                return bc

            for b in range(B):
                xT_s = xp.tile([P, DT, L], f32r, tag="xT")
                nc.sync.dma_start(out=xT_s, in_=xT_ext[b].rearrange("(k p) l -> p k l", p=P))
                x_s = xp.tile([P, LT, D], f32r, tag="x")
                nc.sync.dma_start(out=x_s, in_=x_ext[b].rearrange("(k p) d -> p k d", p=P))
                hTn_s = hp.tile([P, DT, L], f32r, tag="hTn")

                # ================= stage 1 =================
                for lc in range(NLC):
                    ls = slice(lc * LC, (lc + 1) * LC)

                    # ph1: qT[e,l] = W1T^T-contraction over d
                    qT_s = tp.tile([P, DT, LC], f32r, tag="qt")
                    for et in range(DT):
                        ps = pp.tile([P, LC], f32, tag="ps")
                        for dk in range(DT):
                            mm(ps, w1t_s[:, dk, et * P:(et + 1) * P],
                               xT_s[:, dk, ls], dk == 0, dk == DT - 1)
                        nc.scalar.copy(qT_s[:, et, :], ps)

                    # ph2: scoresT[m,l] -> exp -> denom
                    expT_s = tp.tile([P, LT, LC], f32r, tag="exp")
                    ps_d = pp.tile([P, LC], f32, tag="ps")
                    for mt in range(LT):
                        ps = pp.tile([P, LC], f32, tag="ps")
                        for ek in range(DT):
                            mm(ps, xT_s[:, ek, mt * P:(mt + 1) * P],
                               qT_s[:, ek, :], ek == 0, ek == DT - 1)
                        nc.scalar.activation(expT_s[:, mt, :], ps, AF.Exp)
                        mm(ps_d[0:1, :], ones_s, expT_s[:, mt, :],
                           mt == 0, mt == LT - 1)
                    bc1 = bcast_recip(ps_d)

                    # ph3: mixT'[d,l] = sum_m x[m,d] expT[m,l], then normalize
                    mixT_s = tp.tile([P, DT, LC], f32r, tag="mix")
                    ps_m = [pp.tile([P, LC], f32, tag="ps", name=f"psm_{b}_{lc}_{i}")
                            for i in range(DT)]
                    for mk in range(LT):
                        for dt in range(DT):
                            mm(ps_m[dt], x_s[:, mk, dt * P:(dt + 1) * P],
                               expT_s[:, mk, :], mk == 0, mk == LT - 1)
                    for dt in range(DT):
                        nc.vector.tensor_mul(mixT_s[:, dt, :], ps_m[dt], bc1)

                    # ph4: out1T[o,l] = Wo1T-contraction over c=[mix,q]; tanh
                    hT_s = tp.tile([P, DT, LC], f32, tag="ht")
                    for ot in range(DT):
                        ps = pp.tile([P, LC], f32, tag="ps")
                        for ck in range(D2T):
                            rhs = mixT_s[:, ck, :] if ck < DT else qT_s[:, ck - DT, :]
                            mm(ps, wo1t_s[:, ck, ot * P:(ot + 1) * P],
                               rhs, ck == 0, ck == D2T - 1)
                        nc.scalar.activation(hT_s[:, ot, :], ps, AF.Tanh)

                    # ph5: L2 norm over d (partition axis) via ones-matmul
                    hsq_s = tp.tile([P, DT, LC], f32r, tag="qt")
                    for dt in range(DT):
                        nc.scalar.activation(hsq_s[:, dt, :], hT_s[:, dt, :], AF.Square)
                    ps_n = pp.tile([P, LC], f32, tag="ps")
                    for dt in range(DT):
                        mm(ps_n[0:1, :], ones_s, hsq_s[:, dt, :], dt == 0, dt == DT - 1)
                    bc2 = bcast_recip(ps_n, clamp_eps=1e-12)
                    for dt in range(DT):
                        nc.vector.tensor_mul(hTn_s[:, dt, ls], hT_s[:, dt, :], bc2)

                # ph6: transpose hidden_norm back to natural layout.
                # Reuses the now-dead stage-1 qt/ht pool slots for hn.
                hn_a = tp.tile([P, LT // 2, D], f32r, tag="qt", name=f"hna_{b}")
                hn_b = tp.tile([P, LT // 2, D], f32r, tag="ht", name=f"hnb_{b}")

                def hn_nat(lt):
                    return hn_a[:, lt, :] if lt < LT // 2 else hn_b[:, lt - LT // 2, :]

                for lt in range(LT):
                    for dt in range(DT):
                        ps_t = pp.tile([P, P], f32r, tag="ps", name=f"pst_{b}_{lt}_{dt}")
                        nc.tensor.transpose(
                            ps_t, hTn_s[:, dt, lt * P:(lt + 1) * P], ident_s)
                        nc.scalar.copy(hn_nat(lt)[:, dt * P:(dt + 1) * P], ps_t)

                # ================= stage 2 =================
                a2s_s = vp.tile([P, LT], f32, tag="a2s", bufs=1)
                a2p_s = vp.tile([P, LT, NLC], f32, tag="a2p", bufs=1)
                q2r_s = vp.tile([P, D2T, NLC], f32, tag="q2r", bufs=1)
                comb_s = vp.tile([P, C2T], f32, tag="comb", bufs=1)
                scr_s = vp.tile([P, LC], f32, tag="scr", bufs=1)
                for lc in range(NLC):
                    ls = slice(lc * LC, (lc + 1) * LC)

                    def c2T(k, fslice):
                        """combined2T[d2,·] k-tile: [hTn; xT]"""
                        return (hTn_s[:, k, fslice] if k < DT
                                else xT_s[:, k - DT, fslice])

                    # ph7: q2T[e2,l]; column-sum of q2 over l (free axis),
                    # reduced from PSUM (f32) since DVE may not read f32r
                    q2T_s = tp.tile([P, D2T, LC], f32r, tag="q2")
                    for et in range(D2T):
                        ps = pp.tile([P, LC], f32, tag="ps")
                        for dk in range(D2T):
                            mm(ps, w2t_s[:, dk, et * P:(et + 1) * P],
                               c2T(dk, ls), dk == 0, dk == D2T - 1)
                        nc.scalar.copy(q2T_s[:, et, :], ps)
                        nc.vector.tensor_reduce(q2r_s[:, et, lc:lc + 1], ps,
                                                axis=mybir.AxisListType.X,
                                                op=ALU.add)

                    # ph8: scores2T -> exp2 -> denom2.  exp2 is written twice:
                    # f32r for the PE (denominator matmuls) and bf16 for the
                    # DVE column-sum (f32r is PE-only).
                    exp2T_s = tp.tile([P, LT, LC], f32r, tag="exp")
                    exp2b_s = tp.tile([P, LT, LC], mybir.dt.bfloat16, tag="mix",
                                      name=f"exp2b_{b}_{lc}")
                    ps_d = pp.tile([P, LC], f32, tag="ps")
                    for mt in range(LT):
                        ps = pp.tile([P, LC], f32, tag="ps")
                        for ek in range(D2T):
                            mm(ps, c2T(ek, slice(mt * P, (mt + 1) * P)),
                               q2T_s[:, ek, :], ek == 0, ek == D2T - 1)
                        nc.scalar.activation(exp2T_s[:, mt, :], ps, AF.Exp)
                        nc.scalar.activation(exp2b_s[:, mt, :], ps, AF.Exp)
                        mm(ps_d[0:1, :], ones_s, exp2T_s[:, mt, :],
                           mt == 0, mt == LT - 1)
                    bc3 = bcast_recip(ps_d)

                    # ph9: A2 column sums a2s[m] = sum_l exp2T[m,l]/denom[l]
                    for mt in range(LT):
                        nc.vector.tensor_mul(scr_s, exp2b_s[:, mt, :], bc3)
                        nc.vector.tensor_reduce(a2p_s[:, mt, lc:lc + 1], scr_s,
                                                axis=mybir.AxisListType.X,
                                                op=ALU.add)

                # ph10: pooled epilogue.
                # comb = [colsum(A2) @ c, colsum(q2)] as a [4D] column vector.
                nc.vector.tensor_add(comb_s[:, D2T:C2T], q2r_s[:, :, 0],
                                     q2r_s[:, :, 1])
                nc.vector.tensor_add(a2s_s, a2p_s[:, :, 0], a2p_s[:, :, 1])
                a2sr_s = vp.tile([P, LT], f32r, tag="a2sr", bufs=1)
                nc.scalar.copy(a2sr_s, a2s_s)
                # colsum(A2) @ c: two [1,D] row matmuls over the natural-layout
                # halves of c = [hn, x]; rows become comb columns via a DRAM
                # bounce (cross-partition move)
                for half in range(2):
                    ps_mx = pp.tile([1, D], f32, tag="ps", name=f"psmx_{b}_{half}")
                    for mk in range(LT):
                        rhs = hn_nat(mk) if half == 0 else x_s[:, mk, :]
                        mm(ps_mx[0:1, :], a2sr_s[:, mk:mk + 1], rhs,
                           mk == 0, mk == LT - 1)
                    mrow = vp.tile([1, D], f32, tag="mrow", bufs=1,
                                   name=f"mrow_{b}_{half}")
                    nc.scalar.copy(mrow, ps_mx)
                    nc.sync.dma_start(
                        out=mscr_ext[b:b + 1, half * D:(half + 1) * D],
                        in_=mrow[0:1, :])
                nc.sync.dma_start(
                    out=comb_s[:, 0:D2T],
                    in_=mscr_ext[b].rearrange("(k p) -> p k", p=P))
                combr_s = vp.tile([P, C2T], f32r, tag="combr", bufs=1)
                nc.scalar.copy(combr_s, comb_s)
                # emb = comb @ (Wo2^T/L) as a single [1,D] row (the 1/L mean
                # factor is folded into wo2t on the host)
                ps_o = pp.tile([1, D], f32, tag="ps", name=f"pso_{b}")
                for ck in range(C2T):
                    mm(ps_o[0:1, :], combr_s[:, ck:ck + 1], wo2t_s[:, ck, :],
                       ck == 0, ck == C2T - 1)
                orow_s = vp.tile([1, D], f32, tag="orow", bufs=2)
                nc.scalar.copy(orow_s, ps_o)
                nc.sync.dma_start(out=out_ext[b:b + 1, :], in_=orow_s[0:1, :])

    _t1 = _time.time()
    nc.compile()
    print(f"[kernel] tile-trace+schedule {_t1 - _t0:.1f}s, "
          f"bacc compile {_time.time() - _t1:.1f}s", file=sys.stderr, flush=True)
    return nc


def get_nc():
    if "nc" not in _CACHE:
        _CACHE["nc"] = _build_nc()
    return _CACHE["nc"]


def make_in_maps(x, W1, Wo1, W2, Wo2):
    x = np.ascontiguousarray(np.asarray(x, dtype=np.float32))
    xT = np.ascontiguousarray(x.transpose(0, 2, 1))
    w1t = np.ascontiguousarray(np.asarray(W1, np.float32).T)
    wo1t = np.ascontiguousarray(np.asarray(Wo1, np.float32).T)
    w2t = np.ascontiguousarray(np.asarray(W2, np.float32).T)
    # 1/L mean-pooling factor folded into the stage-2 output projection
    wo2t = np.ascontiguousarray(np.asarray(Wo2, np.float32).T) * (1.0 / L)
    ident = np.eye(P, dtype=np.float32)
    onesc = np.ones((P, 1), dtype=np.float32)
    onesr = np.ones((1, P), dtype=np.float32)
    return [
        {"x": x[c * B:(c + 1) * B], "xT": xT[c * B:(c + 1) * B],
         "w1t": w1t, "wo1t": wo1t, "w2t": w2t, "wo2t": wo2t,
         "ident": ident, "onesc": onesc, "onesr": onesr}
        for c in range(NCORES)
    ]


def run(x, W1, Wo1, W2, Wo2, trace=False, **kw):
    from concourse.bass_utils import run_bass_kernel_spmd
    nc = get_nc()
    in_maps = make_in_maps(x, W1, Wo1, W2, Wo2)
    res = run_bass_kernel_spmd(nc, in_maps, core_ids=list(range(NCORES)),
                               trace=trace, **kw)
    out = np.concatenate([res.results[c]["out"] for c in range(NCORES)], axis=0)
    return out.reshape(N_GLOBAL, D, 1, 1), res


def kernel(**inputs):
    out, _ = run(inputs["x"], inputs["W1"], inputs["Wo1"],
                 inputs["W2"], inputs["Wo2"])
    return out

